# revision 67
# baseline (speedup 1.0000x reference)
"""ClsMixAttention Trainium2 Bass kernel.

Strategy: data-parallel over batch across 8 NeuronCores (8 batches/core, no
collectives).  Host-side: tokens permuted to [temporal(128) | cls | spatial(256)]
and x transposed to (C, N) per batch so every matmul streams with the
contraction dim on partitions.  Device-side per batch:
  qT/kT   : (h*d, tok) layout via Wqkv-stationary matmuls  (fp32r)
  v       : (tok, h*d) layout with an interleaved ones-column per head
  S^T     : per head-pair (row-packed K=64 at bases 0/64), keys chunked
            [128,128,128,1]; temporal queries only need key-chunk 0
            (the token permutation makes every attention mask trivial)
  P^T     : exp on ScalarE with fused 0.125 scale, written straight to SBUF
  PV      : M=65 matmuls ([v | ones] stationary) -> PV rows 0:64 and the
            softmax denominator in row 64 of the same psum tile
  norm    : denominators PE-broadcast via an E-matrix matmul, DVE
            reciprocal_approx_fast, multiply fused with the evacuation
  proj    : transposed output (c_out, tok); bias folded into the evac as a
            per-partition tensor_scalar_add
Token dim padded to 386 on-chip (fp32r matmuls need an even moving dim);
the pad column is zeros through qkv, exp(0)=1 through attention, and is
stripped at the output evacuation.  Host gathers (B,768,385) transposed
outputs, untransposes and unpermutes.
"""

import os
import sys
import numpy as np

if "/opt/trn_rl_repo" not in sys.path:
    sys.path.insert(0, "/opt/trn_rl_repo")

# The kernel executes through the axon PJRT backend; a JAX_PLATFORMS=cpu pin
# (used by some harnesses for the reference) would hide the NeuronCores.
if "jax" not in sys.modules and "axon" not in os.environ.get("JAX_PLATFORMS", "axon"):
    os.environ.pop("JAX_PLATFORMS", None)

B, N, C = 64, 385, 768
H, D = 12, 64
NT = 128          # temporal tokens (t_h*t_w*(1+online_size))
P = 128
KC = 6            # C / 128 contraction chunks
SCALE = 0.125     # D ** -0.5
NCORES = 8
BPC = B // NCORES
NW = 386          # padded token width (fp32r needs even moving dim)
KCH = [(0, 128), (128, 256), (256, 384), (384, 385)]   # key/token chunks
# psum-slot reuse for P^T: kc0->slot0, kc1->slot1, kc2->slot2, kc3->slot1
PT_SLOT = [0, 1, 2, 1]

# token permutation: [temporal (orig 1..128), cls (orig 0), spatial (129..384)]
PERM = np.concatenate([np.arange(1, 1 + NT), [0], np.arange(1 + NT, N)])
INV_PERM = np.argsort(PERM)

_CACHE = {}


def build_nc(bpc=BPC):
    import concourse.bacc as bacc
    import concourse.mybir as mybir
    import concourse.tile as tile

    dt = mybir.dt
    f32 = dt.float32
    R = dt.float32r
    AF = mybir.ActivationFunctionType

    nc = bacc.Bacc("TRN2", target_bir_lowering=False, debug=False)
    xt_d = nc.dram_tensor("xt", [bpc, C, N], f32, kind="ExternalInput")
    wqkv_d = nc.dram_tensor("wqkv", [C, 3 * C], f32, kind="ExternalInput")
    wproj_d = nc.dram_tensor("wproj", [C, C], f32, kind="ExternalInput")
    bproj_d = nc.dram_tensor("bproj", [1, C], f32, kind="ExternalInput")
    out_d = nc.dram_tensor("out", [bpc, C, N], f32, kind="ExternalOutput")

    with tile.TileContext(nc) as tc:
        with (
            tc.tile_pool(name="wpool", bufs=1) as wp,
            tc.tile_pool(name="per_b", bufs=1) as bp,
            tc.tile_pool(name="norm", bufs=2) as npool,
            tc.tile_pool(name="outst", bufs=4) as outp,
            tc.tile_pool(name="ps2", bufs=2, space="PSUM") as ps2,
            tc.tile_pool(name="ps1", bufs=4, space="PSUM") as ps1,
        ):
            # -------------- persistent double buffers --------------
            xt_sb = [bp.tile([P, KC * NW], R, tag=f"xt{i}", name=f"xt_sb{i}") for i in range(2)]
            qk_sb = [bp.tile([P, 12 * NW], R, tag=f"qk{i}", name=f"qk_sb{i}") for i in range(2)]
            v_sb = [bp.tile([P, 4 * 780], R, tag=f"v{i}", name=f"v_sb{i}") for i in range(2)]
            pt_sb = [bp.tile([P, 3 * 2 * NW], R, tag=f"pt{i}", name=f"pt_sb{i}") for i in range(2)]
            aot_sb = [bp.tile([P, KC * NW], R, tag=f"aot{i}", name=f"aot_sb{i}") for i in range(2)]
            dn_sb = [bp.tile([33, NW], R, tag=f"dn{i}", name=f"dn_sb{i}") for i in range(2)]
            for t in dn_sb:
                nc.vector.memset(t[:, :].bitcast(dt.uint32), 0)
            for t in pt_sb:
                nc.vector.memset(t[:, :].bitcast(dt.uint32), 0)
            for t in v_sb:
                # ones column after each head's 64 v-columns (denominator trick)
                nc.vector.memset(
                    t[:, :].rearrange("p (c h e) -> p c h e", h=12, e=65)[:, :, :, 64:65].bitcast(dt.uint32),
                    0x3F800000,
                )
            for t in xt_sb:
                # pad column (token 385) stays zero forever
                nc.vector.memset(
                    t[:, :].rearrange("p (k n) -> p k n", n=NW)[:, :, N:NW].bitcast(dt.uint32), 0
                )

            def load_xt(b):
                xb = xt_sb[b % 2]
                xv = xt_d[b].rearrange("(k p) n -> k p n", p=P)
                for kc in range(KC):
                    nc.sync.dma_start(
                        out=xb[:, kc * NW : kc * NW + N], in_=xv[kc].bitcast(R)
                    )

            def qkv(b):
                xb = xt_sb[b % 2]
                qk = qk_sb[b % 2]
                vb = v_sb[b % 2]
                # qT / kT: stationary = Wqkv columns, out (h*d rows, tok)
                for mp in range(6):
                    ps = ps2.tile([P, 1024], f32, tag="st", name="ps_qk")
                    for half in range(2):
                        mc = 2 * mp + half
                        for kc in range(KC):
                            nc.tensor.matmul(
                                ps[:, half * 512 : half * 512 + NW],
                                wqkv_sb[:, kc * 2304 + mc * P : kc * 2304 + (mc + 1) * P],
                                xb[:, kc * NW : (kc + 1) * NW],
                                start=(kc == 0),
                                stop=(kc == KC - 1),
                            )
                    nc.scalar.activation(
                        qk[:, 2 * mp * NW : (2 * mp + 2) * NW],
                        ps[:, :].rearrange("p (s n) -> p s n", n=512)[:, :, 0:NW],
                        AF.Copy,
                        scale=1.0,
                    )
                # v: stationary = xT token chunks, out (tok rows, h*d)
                for tci, (t0, t1) in enumerate(KCH):
                    tw = t1 - t0
                    ps = ps2.tile([P, 1024], f32, tag="st", name="ps_v")
                    for nh in range(2):
                        for kc in range(KC):
                            nc.tensor.matmul(
                                ps[0:tw, nh * 512 : nh * 512 + 384],
                                xb[:, kc * NW + t0 : kc * NW + t1],
                                wqkv_sb[:, kc * 2304 + 1536 + nh * 384 : kc * 2304 + 1536 + (nh + 1) * 384],
                                start=(kc == 0),
                                stop=(kc == KC - 1),
                            )
                    # two sequential DVE copies: heads 0-5 land first so early
                    # pairs' PV can start half an evac sooner
                    nc.vector.tensor_copy(
                        vb[0:tw, tci * 780 : tci * 780 + 390]
                        .rearrange("p (h e) -> p h e", e=65)[:, :, 0:64],
                        ps[0:tw, 0:384].rearrange("p (h e) -> p h e", e=64),
                    )
                    nc.vector.tensor_copy(
                        vb[0:tw, tci * 780 + 390 : (tci + 1) * 780]
                        .rearrange("p (h e) -> p h e", e=65)[:, :, 0:64],
                        ps[0:tw, 512:896].rearrange("p (h e) -> p h e", e=64),
                    )

            def attention(b):
                qk = qk_sb[b % 2]
                vb = v_sb[b % 2]
                aot = aot_sb[b % 2]
                for p in range(6):
                    pts = pt_sb[p % 2]
                    qoff = p * NW
                    koff = (6 + p) * NW
                    pva = ps1.tile([P, 512], f32, tag="u", name="ps_pva")
                    pvb = ps1.tile([P, 512], f32, tag="u", name="ps_pvb")
                    pvs = (pva, pvb)
                    for kci, (k0, k1) in enumerate(KCH):
                        kw = k1 - k0
                        base = PT_SLOT[kci] * 2 * NW
                        # query col range: kc0 serves all queries, others cs only
                        q0, qn = (0, NW) if kci == 0 else (128, 258)
                        st = ps2.tile([P, 1024], f32, tag="st", name="ps_st")
                        for hh in range(2):
                            nc.tensor.matmul(
                                st[0:kw, hh * 512 + q0 : hh * 512 + q0 + qn],
                                qk[hh * 64 : (hh + 1) * 64, koff + k0 : koff + k1],
                                qk[hh * 64 : (hh + 1) * 64, qoff + q0 : qoff + q0 + qn],
                                start=True,
                                stop=True,
                            )
                        nc.scalar.activation(
                            pts[0:kw, base : base + 2 * NW].rearrange("p (s n) -> p s n", n=NW)[:, :, q0 : q0 + qn],
                            st[0:kw, :].rearrange("p (s n) -> p s n", n=512)[:, :, q0 : q0 + qn],
                            AF.Exp,
                            scale=SCALE,
                        )
                        for hh in range(2):
                            # lhsT = [v_h | ones] (65 cols) -> rows 0:64 PV, row 64 denom.
                            # kc1-3 only contribute to cs queries (cols 128:386);
                            # temporal cols are kc0-only, so skip their zeros here.
                            nc.tensor.matmul(
                                pvs[hh][0:65, q0 : q0 + qn],
                                vb[0:kw, kci * 780 + (2 * p + hh) * 65 : kci * 780 + (2 * p + hh) * 65 + 65],
                                pts[0:kw, base + hh * NW + q0 : base + hh * NW + q0 + qn],
                                start=(kci == 0),
                                stop=(kci == 3),
                            )
                    # normalize: PE-broadcast denoms, reciprocal, multiply (=evac)
                    dn2 = dn_sb[p % 2]
                    nc.vector.tensor_copy(dn2[0:1, :], pva[64:65, 0:NW])
                    nc.scalar.activation(dn2[32:33, :], pvb[64:65, 0:NW], AF.Copy, scale=1.0)
                    bc = ps1.tile([P, 512], f32, tag="u", name="ps_bc")
                    nc.tensor.matmul(bc[:, 0:NW], e_bc[:, :], dn2[:, :], start=True, stop=True)
                    rb = npool.tile([P, NW], f32, tag="rb", name="rb", bufs=6)
                    nc.vector.reciprocal_approx_fast(rb[:, :], bc[:, 0:NW])
                    nc.vector.tensor_mul(
                        aot[0:64, p * NW : (p + 1) * NW], pva[0:64, 0:NW], rb[0:64, :]
                    )
                    nc.vector.tensor_mul(
                        aot[64:128, p * NW : (p + 1) * NW], pvb[0:64, 0:NW], rb[64:128, :]
                    )

            def proj(b):
                aot = aot_sb[b % 2]
                for mc in range(KC):
                    ps = ps1.tile([P, 512], f32, tag="u", name="ps_u")
                    for kc in range(KC):
                        nc.tensor.matmul(
                            ps[:, 0:NW],
                            wproj_sb[:, kc * C + mc * P : kc * C + (mc + 1) * P],
                            aot[:, kc * NW : (kc + 1) * NW],
                            start=(kc == 0),
                            stop=(kc == KC - 1),
                        )
                    ot = outp.tile([P, N], f32, tag="ot", name="ot")
                    nc.vector.tensor_scalar_add(ot[:, :], ps[:, 0:N], bproj_pc[:, mc : mc + 1])
                    nc.sync.dma_start(
                        out=out_d[b].rearrange("(k p) n -> k p n", p=P)[mc],
                        in_=ot[:, :],
                    )

            load_xt(0)
            # ---------------- weights ----------------
            # quarter-major order: the first-needed Wqkv columns land first
            wqkv_sb = wp.tile([P, KC * 3 * C], R)       # [128, 13824]
            wv = wqkv_d[:].rearrange("(k p) c -> k p c", p=P)
            for q4 in range(4):
                for kc in range(KC):
                    nc.sync.dma_start(
                        out=wqkv_sb[:, kc * 2304 + q4 * 576 : kc * 2304 + (q4 + 1) * 576],
                        in_=wv[kc, :, q4 * 576 : (q4 + 1) * 576].bitcast(R),
                    )
            wproj_sb = wp.tile([P, KC * C], R)          # [128, 4608]
            wpv = wproj_d[:].rearrange("(k p) c -> k p c", p=P)
            for kc in range(KC):
                nc.sync.dma_start(
                    out=wproj_sb[:, kc * C : (kc + 1) * C], in_=wpv[kc].bitcast(R)
                )
            # bias in partition-major layout: value (mc*128+p) at [p, mc]
            bproj_pc = wp.tile([P, KC], f32)
            nc.sync.dma_start(
                out=bproj_pc[:, :],
                in_=bproj_d[0].rearrange("(k p) -> p k", p=P),
            )

            # E-matrix for PE denominator broadcast: rows 0/32 select head A/B
            e_bc = wp.tile([33, P], R)
            nc.vector.memset(e_bc[:, :].bitcast(dt.uint32), 0)
            nc.vector.memset(e_bc[0:1, 0:64].bitcast(dt.uint32), 0x3F800000)
            nc.vector.memset(e_bc[32:33, 64:128].bitcast(dt.uint32), 0x3F800000)

            for b in range(bpc):
                if b + 1 < bpc:
                    load_xt(b + 1)
                if b > 0:
                    proj(b - 1)
                qkv(b)
                attention(b)
            proj(bpc - 1)

    nc.compile()
    return nc


def _prep_inputs(x, Wqkv, Wproj, bproj):
    x = np.asarray(x, dtype=np.float32)
    xt = np.ascontiguousarray(x[:, PERM, :].transpose(0, 2, 1))   # (B, C, N)
    wqkv = np.ascontiguousarray(np.asarray(Wqkv, dtype=np.float32))
    wproj = np.ascontiguousarray(np.asarray(Wproj, dtype=np.float32))
    bp = np.ascontiguousarray(np.asarray(bproj, dtype=np.float32).reshape(1, C))
    in_maps = []
    for i in range(NCORES):
        in_maps.append({
            "xt": np.ascontiguousarray(xt[i * BPC : (i + 1) * BPC]),
            "wqkv": wqkv,
            "wproj": wproj,
            "bproj": bp,
        })
    return in_maps


def _postprocess(results):
    outs = [results[i]["out"] for i in range(NCORES)]          # (BPC, C, N) each
    out_t = np.concatenate(outs, axis=0)                       # (B, C, N)
    out = out_t.transpose(0, 2, 1)[:, INV_PERM, :]             # (B, N, C)
    return np.ascontiguousarray(out)


def run(inputs, trace=False):
    from concourse.bass_utils import run_bass_kernel_spmd

    if "nc" not in _CACHE:
        _CACHE["nc"] = build_nc(BPC)
    nc = _CACHE["nc"]
    in_maps = _prep_inputs(inputs["x"], inputs["Wqkv"], inputs["Wproj"], inputs["bproj"])
    res = run_bass_kernel_spmd(nc, in_maps, list(range(NCORES)), trace=trace)
    return _postprocess(res.results), res


def kernel(x, Wqkv, Wproj, bproj, t_h=8, t_w=8, s_h=16, s_w=16, online_size=1, num_heads=12, **_):
    assert int(t_h) * int(t_w) * (1 + int(online_size)) == NT
    assert int(s_h) * int(s_w) == N - 1 - NT
    assert int(num_heads) == H
    out, _res = run({"x": x, "Wqkv": Wqkv, "Wproj": Wproj, "bproj": bproj})
    return out
